# revision 10
# baseline (speedup 1.0000x reference)
"""MoE-routed conv kernel (Channel_Embedding ablation) for 8 trn2 NeuronCores.

Math (see reference):
  gates  = top2-renormalized softmax( x[:, :, -6:-1].reshape(B, D*5) @ w_gate )
  h      = tanh(conv1d(x, conv1_w, VALID) + conv1_b)            # [B, OC, L-2]
  out    = conv1d(h, conv2_w, 1x1) + conv2_b                    # [B, OC*E, L-2]
  y[b,oc,t] = sum_e gates[b,e] * out[b, oc*E+e, t]

Key algebraic fold: the expert combine commutes with the 1x1 conv, so per
batch element
  W_eff[b][oc, ic] = sum_e gates[b,e] * conv2_w[oc*E+e, ic, 0]
  b_eff[b][oc]     = sum_e gates[b,e] * conv2_b[oc*E+e]
  y[b] = W_eff[b] @ h[b] + b_eff[b]

Sharding: data-parallel over batch B=32 across 8 cores (4 each); weights
replicated.

v2 layout (all 128 partitions, bf16 hot path):
  x ships from host as bf16; xf[64q + d, 4096p + c] = x[2q+p, d, c].
  Conv matmuls are bf16 with block-diag weights over q (K=128 = 2 batches
  x 64 ch); pair p=1 writes PSUM partitions 64:128 via the matmul tile
  position, so each 512-position tile accumulates ONE [128, 512] PSUM
  image covering all 4 batches -> one tanh, one 128x128 block-diag
  combine matmul, one bias-add drain. y accumulates in SBUF as bf16 and
  is upcast on the host (gate is 2e-2; bf16 path lands ~3e-3).

Gating is strict fp32 (expert top-2 selection must match the reference):
the 5-column gating window of x rides inside the fp32 const image, so
gating never touches the bf16 x.

DMA: consts first (sync), then x in 4 column chunks (first on sync for
latency, rest on gpsimd whose SWDGE issue cost is ~20x cheaper than
sync's HWDGE config); chunks overlap by 2 columns so a position tile
only depends on its own chunk. y stores in 2 half-length chunks.
"""

from contextlib import ExitStack

import numpy as np

import concourse.bacc as bacc
import concourse.mybir as mybir
import concourse.tile as tile
from concourse import bass_utils

B, D, L = 32, 64, 4096
E, TOPK, OC = 8, 2, 32
LP = L - 2  # 4094 valid conv outputs
NCORES = 8
NB = B // NCORES  # batch elements per core
TS = 512  # position tile (one PSUM bank of fp32)
NT = (LP + TS - 1) // TS

FAST_DT = mybir.dt.float32r  # fp32 bits, 1 cycle/row on PE at N>=256
BF16 = mybir.dt.bfloat16

# f32 constants image [128, NCF]: gating weights (dup in both q halves),
# conv1 bias tiled 4x, and the per-core fp32 gating window of x.
C_WG = 0  # rows 0:64 AND 64:128, [*, 40], col = t*8 + e
C_B1P = C_WG + 5 * E  # [128, 1] conv1 bias tiled 4x (partition p,q,oc)
C_GW = C_B1P + 1  # [128, 10] gwin[64q+d, 5p+t] = x[2q+p, d, L-6+t]
NCF = C_GW + 10
NW1T = 3 * 2 * OC  # bf16 image: block-diag conv1 weights [128, 192]
# fp32r conv2 image [8, 1056]: c2w[e, ic*32+oc], c2b[e, oc]
C2_W, C2_B, NC2 = 0, OC * OC, OC * OC + OC

_CACHE: dict = {}


def _softmax_top2(nc, sm, lg, f32, AX, OP, AF, q):
    """Per-half gating: lg [2, E] logits (PSUM) -> gtr [32, 32] in SBUF.

    gates = (e >= m2) * e / (m1 + m2 + 1e-6 * sum(e)), e = exp(logits) --
    identical to softmax -> top2 -> vk/(sum vk + 1e-6) in exact arithmetic.
    """
    e_sb = sm.tile([2, E], f32, name=f"e_sb{q}")
    nc.scalar.activation(e_sb[:], lg[:], AF.Exp)
    m1 = sm.tile([2, 1], f32, name=f"m1_{q}")
    nc.vector.reduce_max(m1[:], e_sb[:], axis=AX.X)
    lt = sm.tile([2, E], f32, name=f"lt{q}")
    nc.vector.tensor_scalar(lt[:], e_sb[:], m1[:], None, op0=OP.is_lt)
    emsk = sm.tile([2, E], f32, name=f"emsk{q}")
    nc.vector.tensor_mul(emsk[:], lt[:], e_sb[:])  # e with the max zeroed
    m2 = sm.tile([2, 1], f32, name=f"m2_{q}")
    nc.vector.reduce_max(m2[:], emsk[:], axis=AX.X)
    den3 = sm.tile([2, 1], f32, name=f"den3{q}")
    nc.vector.tensor_add(den3[:], m1[:], m2[:])
    rcp = sm.tile([2, 1], f32, name=f"rcp{q}")
    nc.vector.reciprocal(rcp[:], den3[:])
    ge = sm.tile([2, E], f32, name=f"ge{q}")
    nc.vector.tensor_scalar(ge[:], e_sb[:], m2[:], None, op0=OP.is_ge)
    gnum = sm.tile([2, E], f32, name=f"gnum{q}")
    nc.vector.tensor_mul(gnum[:], ge[:], e_sb[:])
    gpad = sm.tile([32, 32], f32, name=f"gpad{q}")
    nc.vector.memset(gpad[:], 0.0)
    nc.vector.tensor_scalar(gpad[0:2, 0:E], gnum[:], rcp[:], None, op0=OP.mult)
    gtr = sm.tile([32, 32], f32, name=f"gtr{q}")
    nc.vector.transpose(gtr[:], gpad[:])  # 32x32 block transpose
    return gtr  # gtr[0:E, 0:2] = gates.T for batches {2q, 2q+1}


def _emit(ctx, tc, nc, x_d, cf_d, w1_d, c2r_d, y_d):
    f32 = mybir.dt.float32
    AF = mybir.ActivationFunctionType
    AX = mybir.AxisListType
    OP = mybir.AluOpType

    const = ctx.enter_context(tc.tile_pool(name="const", bufs=1))
    sm = ctx.enter_context(tc.tile_pool(name="sm", bufs=1))
    hsb = ctx.enter_context(tc.tile_pool(name="hsb", bufs=4))
    psum_h = ctx.enter_context(tc.tile_pool(name="ph", bufs=3, space="PSUM"))
    psum_o = ctx.enter_context(tc.tile_pool(name="po", bufs=3, space="PSUM"))
    psum_s = ctx.enter_context(tc.tile_pool(name="ps", bufs=2, space="PSUM"))
    dram = ctx.enter_context(tc.tile_pool(name="dram", bufs=1, space="DRAM"))

    # ---- consts first: gating + conv weights (small, gate everything)
    cf = const.tile([128, NCF], f32)
    nc.sync.dma_start(cf[:], cf_d.ap(), max_dma_last_dim=NCF)
    w1t = const.tile([128, NW1T], BF16)
    nc.sync.dma_start(w1t[:], w1_d.ap(), max_dma_last_dim=NW1T)
    c2r = const.tile([E, NC2], FAST_DT)
    nc.sync.dma_start(c2r[:], c2r_d.ap().bitcast(FAST_DT), max_dma_last_dim=NC2)
    c2w = c2r[0:E, C2_W : C2_W + OC * OC]
    c2b = c2r[0:E, C2_B : C2_B + OC]
    b1p = cf[:, C_B1P : C_B1P + 1]

    # ---- x image in bf16, two DISJOINT column halves per q (disjoint so
    # the chunk DMAs carry no WAW deps and stream fully in parallel; 4KB
    # contiguous runs for DMA-engine efficiency). First half on sync for
    # latency, second on gpsimd.
    xf = const.tile([2 * D, 2 * L], BF16)
    xv = x_d.ap().rearrange("(q p) d c -> q d p c", q=2)

    def load_chunk(eng, a0, a1):
        for q in range(2):
            eng.dma_start(
                xf[D * q : D * q + D, :].rearrange("d (p c) -> d p c", p=2)[
                    :, :, a0:a1
                ],
                xv[q : q + 1, :, :, a0:a1],
            )

    load_chunk(nc.sync, 0, 2050)
    load_chunk(nc.gpsimd, 2050, 4096)

    # ---- ACT table warmup (exp/tanh share one table set; load it early)
    warm = sm.tile([1, 8], f32)
    nc.vector.memset(warm[:], 0.0)
    warm2 = sm.tile([1, 8], f32)
    nc.scalar.activation(warm2[:], warm[:], AF.Exp)

    # ---- PE warmup: dummy matmuls on a memset tile (no data deps) to
    # start the PE p-state ramp during the const load window.
    wsrc = sm.tile([128, 512], BF16)
    nc.vector.memset(wsrc[:], 0.0)
    wup = psum_s.tile([64, 512], f32, tag="s")
    for _ in range(3):
        nc.tensor.matmul(wup[:], wsrc[:, 0:64], wsrc[:], start=True, stop=True)

    # ---- gating from the fp32 window in cf: per q-half (batches {2q,2q+1})
    # logits[b, e] = sum_{d,t} gwin[64q+d, 5p+t] * w_gate[d*5+t, e]
    gwin = cf[:, C_GW : C_GW + 10].rearrange("d (p t) -> d t p", p=2)
    gtrs = []
    for q in range(2):
        lg = psum_s.tile([2, E], f32, tag="s", name=f"lg{q}")
        for t in range(5):
            nc.tensor.matmul(
                lg[:],
                gwin[D * q : D * q + D, t : t + 1, :],
                cf[D * q : D * q + D, C_WG + E * t : C_WG + E * t + E],
                start=(t == 0),
                stop=(t == 4),
            )
        gtrs.append(_softmax_top2(nc, sm, lg, f32, AX, OP, AF, q))
    gT = sm.tile([E, NB], FAST_DT)  # col b = 2q + p
    for q in range(2):
        nc.vector.tensor_copy(gT[:, 2 * q : 2 * q + 2], gtrs[q][0:E, 0:2])

    # ---- b_eff: bT[oc, b] = sum_e c2b[e, oc] * gates[b, e] (PE transpose),
    # then scatter to beff128[64p + 32q + oc] = b_eff[2q+p][oc] (4 tiny
    # SBUF->SBUF DMAs; engines cannot shift partitions).
    wp3 = psum_s.tile([OC, NB], f32, tag="s")
    nc.tensor.matmul(wp3[:], c2b[:], gT[:], start=True, stop=True)
    bT = sm.tile([OC, NB], f32)
    nc.vector.tensor_copy(bT[:], wp3[:])
    beff = const.tile([128, 1], f32)
    for j in range(4):
        p, q = j // 2, j % 2
        nc.gpsimd.dma_start(
            beff[32 * j : 32 * j + 32, :], bT[0:OC, 2 * q + p : 2 * q + p + 1]
        )

    # ---- W_eff[b] = gates[b] @ c2: weff[b, ic*32+oc] in fp32, cast bf16
    weff = sm.tile([NB, OC * OC], BF16)
    wp1 = psum_s.tile([NB, 512], f32, tag="s")
    nc.tensor.matmul(wp1[:], gT[:], c2w[:, 0:512], start=True, stop=True)
    nc.vector.tensor_copy(weff[:, 0:512], wp1[:])
    wp2 = psum_s.tile([NB, 512], f32, tag="s")
    nc.tensor.matmul(wp2[:], gT[:], c2w[:, 512:1024], start=True, stop=True)
    nc.scalar.copy(weff[:, 512:1024], wp2[:])

    # Block-diag combine weights for the single 128x128 combine matmul:
    #   weT[64p + 32q + ic, 64p + 32q + oc] = W_eff[2q+p][oc, ic]
    # via a DRAM bounce (SBUF APs must not cross partitions mid-dim; the
    # DRAM-side strided reads are fine), one scatter DMA per 32x32 block.
    wscr = dram.tile([NB, OC * OC], BF16)
    nc.scalar.dma_start(wscr[:], weff[:], max_dma_last_dim=OC * OC)
    weT = const.tile([128, 128], BF16)
    nc.vector.memset(weT[:], 0.0)
    for j in range(4):
        p, q = j // 2, j % 2
        b = 2 * q + p
        nc.scalar.dma_start(
            weT[32 * j : 32 * j + 32, 32 * j : 32 * j + 32],
            wscr[b : b + 1, :].rearrange("b (ic oc) -> (b ic) oc", ic=OC),
        )

    # persistent output image: yb[64p + 32q + oc, c] = y[2q+p, oc, c], bf16
    yb = const.tile([128, LP], BF16)

    # ---- main loop over position tiles
    for t in range(NT):
        c0 = t * TS
        n = min(TS, LP - c0)
        hp = psum_h.tile([128, TS], f32, tag="hp")
        for k in range(3):
            for p in range(2):
                nc.tensor.matmul(
                    hp[64 * p : 64 * p + 64, 0:n],
                    w1t[:, 64 * k : 64 * k + 64],
                    xf[:, L * p + c0 + k : L * p + c0 + k + n],
                    start=(k == 0),
                    stop=(k == 2),
                )
        hs = hsb.tile([128, TS], BF16, tag="hs")
        nc.scalar.activation(hs[:, 0:n], hp[:, 0:n], AF.Tanh, bias=b1p, scale=1.0)
        op_ = psum_o.tile([128, TS], f32, tag="op")
        nc.tensor.matmul(op_[:, 0:n], weT[:], hs[:, 0:n], start=True, stop=True)
        # PSUM drain + b_eff add + bf16 cast, alternating DVE / ACT
        if t % 2 == 0:
            nc.vector.tensor_scalar(
                yb[:, c0 : c0 + n], op_[:, 0:n], beff[:], None, op0=OP.add
            )
        else:
            nc.scalar.add(yb[:, c0 : c0 + n], op_[:, 0:n], beff[:])
        if t == NT // 2 - 1:
            _store_half(nc, [nc.gpsimd] * 4, y_d, yb, 0, LP // 2)
    _store_half(nc, [nc.sync, nc.sync, nc.gpsimd, nc.gpsimd], y_d, yb, LP // 2, LP)


def _store_half(nc, engs, y_d, yb, a0, a1):
    # batch b = 2q+p lives at partition block j = 2p+q
    for b in range(NB):
        p, q = b % 2, b // 2
        j = 2 * p + q
        engs[b].dma_start(
            y_d.ap()[b, :, a0:a1],
            yb[32 * j : 32 * j + 32, a0:a1],
            max_dma_last_dim=a1 - a0,
        )


def _build():
    if "nc" in _CACHE:
        return _CACHE["nc"]
    nc = bacc.Bacc(
        "TRN2",
        target_bir_lowering=False,
        debug=False,
        num_devices=NCORES,
        detect_race_conditions=False,
    )
    f32 = mybir.dt.float32
    x_d = nc.dram_tensor("x", [NB, D, L], BF16, kind="ExternalInput")
    cf_d = nc.dram_tensor("cf", [128, NCF], f32, kind="ExternalInput")
    w1_d = nc.dram_tensor("w1", [128, NW1T], BF16, kind="ExternalInput")
    c2r_d = nc.dram_tensor("c2r", [E, NC2], f32, kind="ExternalInput")
    y_d = nc.dram_tensor("y", [NB, OC, LP], BF16, kind="ExternalOutput")

    with tile.TileContext(nc) as tc:
        with ExitStack() as ctx:
            _emit(ctx, tc, nc, x_d, cf_d, w1_d, c2r_d, y_d)
    nc.compile()
    _CACHE["nc"] = nc
    return nc


def _prep_weights(w_gate, conv1_w, conv1_b, conv2_w, conv2_b):
    import ml_dtypes

    bf16 = ml_dtypes.bfloat16
    w_gate = np.asarray(w_gate, np.float32)
    conv1_w = np.asarray(conv1_w, np.float32)
    conv1_b = np.asarray(conv1_b, np.float32)
    conv2_w = np.asarray(conv2_w, np.float32)
    conv2_b = np.asarray(conv2_b, np.float32)
    # bf16 image: block-diagonal conv1 weights for 2-batch-stacked matmuls
    w1 = np.zeros((128, NW1T), bf16)
    wkt = conv1_w.transpose(1, 2, 0).astype(bf16)  # [d, k, oc]
    for k in range(3):
        w1[0:D, 64 * k : 64 * k + OC] = wkt[:, k, :]
        w1[D : 2 * D, 64 * k + OC : 64 * k + 2 * OC] = wkt[:, k, :]
    # f32 image (gating window filled per core in _run)
    cf = np.zeros((128, NCF), np.float32)
    wgr = w_gate.reshape(D, 5 * E)
    cf[0:D, C_WG : C_WG + 5 * E] = wgr
    cf[D : 2 * D, C_WG : C_WG + 5 * E] = wgr
    cf[:, C_B1P] = np.tile(conv1_b, 4)
    # fp32r conv2 image: c2w[e, ic*32+oc] = conv2_w[oc*8+e, ic, 0]
    c2 = np.zeros((E, NC2), np.float32)
    c2[:, C2_W : C2_W + OC * OC] = (
        conv2_w[:, :, 0].reshape(OC, E, OC).transpose(1, 2, 0).reshape(E, OC * OC)
    )
    c2[:, C2_B : C2_B + OC] = conv2_b.reshape(OC, E).T
    return np.ascontiguousarray(w1), cf, np.ascontiguousarray(c2)


def _run(x, w_gate, conv1_w, conv1_b, conv2_w, conv2_b, **spmd_kwargs):
    import ml_dtypes

    bf16 = ml_dtypes.bfloat16
    x = np.asarray(x, np.float32)
    assert x.shape == (B, D, L), x.shape
    w1, cf, c2 = _prep_weights(w_gate, conv1_w, conv1_b, conv2_w, conv2_b)
    xb = np.ascontiguousarray(x.astype(bf16))
    nc = _build()
    in_maps = []
    for i in range(NCORES):
        xs = x[NB * i : NB * (i + 1)]  # fp32 shard for the gating window
        cfi = cf.copy()
        # gwin[64q+d, 5p+t] = x[2q+p, d, L-6+t]
        gw = xs[:, :, L - 6 : L - 1].reshape(2, 2, D, 5)  # [q, p, d, t]
        cfi[:, C_GW : C_GW + 10] = (
            gw.transpose(0, 2, 1, 3).reshape(2 * D, 10)
        )
        in_maps.append(
            {
                "x": np.ascontiguousarray(xb[NB * i : NB * (i + 1)]),
                "cf": cfi,
                "w1": w1,
                "c2r": c2,
            }
        )
    res = bass_utils.run_bass_kernel_spmd(
        nc, in_maps, core_ids=list(range(NCORES)), **spmd_kwargs
    )
    # y[64p + 32q + oc] partition layout maps back to batch b = 2q + p
    y = np.concatenate([r["y"] for r in res.results], axis=0)
    return np.ascontiguousarray(y.astype(np.float32)), res


def kernel(x, w_gate, conv1_w, conv1_b, conv2_w, conv2_b):
    y, _ = _run(x, w_gate, conv1_w, conv1_b, conv2_w, conv2_b)
    return y


# revision 15
# speedup vs baseline: 1.1092x; 1.1092x over previous
"""MoE-routed conv kernel (Channel_Embedding ablation) for 8 trn2 NeuronCores.

Math (see reference):
  gates  = top2-renormalized softmax( x[:, :, -6:-1].reshape(B, D*5) @ w_gate )
  h      = tanh(conv1d(x, conv1_w, VALID) + conv1_b)            # [B, OC, L-2]
  out    = conv1d(h, conv2_w, 1x1) + conv2_b                    # [B, OC*E, L-2]
  y[b,oc,t] = sum_e gates[b,e] * out[b, oc*E+e, t]

Key algebraic fold: the expert combine commutes with the 1x1 conv, so per
batch element
  W_eff[b][oc, ic] = sum_e gates[b,e] * conv2_w[oc*E+e, ic, 0]
  b_eff[b][oc]     = sum_e gates[b,e] * conv2_b[oc*E+e]
  y[b] = W_eff[b] @ h[b] + b_eff[b]

Sharding: data-parallel over batch B=32 across 8 cores (4 each); weights
replicated.

v2 layout (all 128 partitions, bf16 hot path):
  x ships from host as bf16; xf[64q + d, 4096p + c] = x[2q+p, d, c].
  Conv matmuls are bf16 with block-diag weights over q (K=128 = 2 batches
  x 64 ch); pair p=1 writes PSUM partitions 64:128 via the matmul tile
  position, so each 512-position tile accumulates ONE [128, 512] PSUM
  image covering all 4 batches -> one tanh, one 128x128 block-diag
  combine matmul, one bias-add drain. y accumulates in SBUF as bf16 and
  is upcast on the host (gate is 2e-2; bf16 path lands ~3e-3).

Gating is strict fp32 (expert top-2 selection must match the reference):
the 5-column gating window of x rides inside the fp32 const image, so
gating never touches the bf16 x.

DMA: consts first (sync), then x in 4 column chunks (first on sync for
latency, rest on gpsimd whose SWDGE issue cost is ~20x cheaper than
sync's HWDGE config); chunks overlap by 2 columns so a position tile
only depends on its own chunk. y stores in 2 half-length chunks.
"""

from contextlib import ExitStack

import numpy as np

import concourse.bacc as bacc
import concourse.mybir as mybir
import concourse.tile as tile
from concourse import bass_utils

B, D, L = 32, 64, 4096
E, TOPK, OC = 8, 2, 32
LP = L - 2  # 4094 valid conv outputs
NCORES = 8
NB = B // NCORES  # batch elements per core
TS = 512  # position tile (one PSUM bank of fp32)
NT = (LP + TS - 1) // TS

FAST_DT = mybir.dt.float32r  # fp32 bits, 1 cycle/row on PE at N>=256
BF16 = mybir.dt.bfloat16

# f32 constants image [128, NCF]: gating weights (dup in both q halves),
# conv1 bias tiled 4x, and the per-core fp32 gating window of x.
C_WG = 0  # rows 0:64 AND 64:128, [*, 40], col = t*8 + e
C_B1P = C_WG + 5 * E  # [128, 1] conv1 bias tiled 4x (partition p,q,oc)
C_GW = C_B1P + 1  # [128, 10] gwin[64q+d, 5p+t] = x[2q+p, d, L-6+t]
NCF = C_GW + 10
NW1T = 3 * 2 * OC  # bf16 image: block-diag conv1 weights [128, 192]
# fp32r conv2 image [8, 1056]: c2w[e, ic*32+oc], c2b[e, oc]
C2_W, C2_B, NC2 = 0, OC * OC, OC * OC + OC

_CACHE: dict = {}


def _softmax_top2(nc, sm, lg, f32, AX, OP, AF, q):
    """Per-half gating: lg [2, E] logits (PSUM) -> gtr [32, 32] in SBUF.

    gates = (e >= m2) * e / (m1 + m2 + 1e-6 * sum(e)), e = exp(logits) --
    identical to softmax -> top2 -> vk/(sum vk + 1e-6) in exact arithmetic.
    """
    e_sb = sm.tile([2, E], f32, name=f"e_sb{q}")
    nc.scalar.activation(e_sb[:], lg[:], AF.Exp)
    m1 = sm.tile([2, 1], f32, name=f"m1_{q}")
    nc.vector.reduce_max(m1[:], e_sb[:], axis=AX.X)
    lt = sm.tile([2, E], f32, name=f"lt{q}")
    nc.vector.tensor_scalar(lt[:], e_sb[:], m1[:], None, op0=OP.is_lt)
    emsk = sm.tile([2, E], f32, name=f"emsk{q}")
    nc.vector.tensor_mul(emsk[:], lt[:], e_sb[:])  # e with the max zeroed
    m2 = sm.tile([2, 1], f32, name=f"m2_{q}")
    nc.vector.reduce_max(m2[:], emsk[:], axis=AX.X)
    den3 = sm.tile([2, 1], f32, name=f"den3{q}")
    nc.vector.tensor_add(den3[:], m1[:], m2[:])
    rcp = sm.tile([2, 1], f32, name=f"rcp{q}")
    nc.vector.reciprocal(rcp[:], den3[:])
    ge = sm.tile([2, E], f32, name=f"ge{q}")
    nc.vector.tensor_scalar(ge[:], e_sb[:], m2[:], None, op0=OP.is_ge)
    gnum = sm.tile([2, E], f32, name=f"gnum{q}")
    nc.vector.tensor_mul(gnum[:], ge[:], e_sb[:])
    gpad = sm.tile([32, 32], f32, name=f"gpad{q}")
    nc.vector.memset(gpad[:], 0.0)
    nc.vector.tensor_scalar(gpad[0:2, 0:E], gnum[:], rcp[:], None, op0=OP.mult)
    gtr = sm.tile([32, 32], f32, name=f"gtr{q}")
    nc.vector.transpose(gtr[:], gpad[:])  # 32x32 block transpose
    return gtr  # gtr[0:E, 0:2] = gates.T for batches {2q, 2q+1}


def _emit(ctx, tc, nc, x_d, cf_d, w1_d, c2r_d, y_d):
    f32 = mybir.dt.float32
    AF = mybir.ActivationFunctionType
    AX = mybir.AxisListType
    OP = mybir.AluOpType

    const = ctx.enter_context(tc.tile_pool(name="const", bufs=1))
    sm = ctx.enter_context(tc.tile_pool(name="sm", bufs=1))
    hsb = ctx.enter_context(tc.tile_pool(name="hsb", bufs=4))
    psum_h = ctx.enter_context(tc.tile_pool(name="ph", bufs=3, space="PSUM"))
    psum_o = ctx.enter_context(tc.tile_pool(name="po", bufs=3, space="PSUM"))
    psum_s = ctx.enter_context(tc.tile_pool(name="ps", bufs=2, space="PSUM"))
    dram = ctx.enter_context(tc.tile_pool(name="dram", bufs=1, space="DRAM"))

    # ---- consts on the scalar queue (kept free of bulk x traffic so the
    # W_eff bounce DMAs later land with low latency)
    cf = const.tile([128, NCF], f32)
    nc.scalar.dma_start(cf[:], cf_d.ap(), max_dma_last_dim=NCF)
    w1t = const.tile([128, NW1T], BF16)
    nc.scalar.dma_start(w1t[:], w1_d.ap(), max_dma_last_dim=NW1T)
    c2r = const.tile([E, NC2], FAST_DT)
    nc.scalar.dma_start(c2r[:], c2r_d.ap().bitcast(FAST_DT), max_dma_last_dim=NC2)
    c2w = c2r[0:E, C2_W : C2_W + OC * OC]
    c2b = c2r[0:E, C2_B : C2_B + OC]
    b1p = cf[:, C_B1P : C_B1P + 1]

    # ---- x image in bf16: 4 DISJOINT column chunks (no WAW deps between
    # chunk DMAs), q=0 pieces on sync, q=1 on gpsimd so the two hardware
    # queues stream in parallel; the last chunk goes on scalar AFTER the
    # W_eff bounce (emitted later) to add a third queue.
    xf = const.tile([2 * D, 2 * L], BF16)
    xv = x_d.ap().rearrange("(q p) d c -> q d p c", q=2)

    def load_chunk_q(eng, q, a0, a1):
        eng.dma_start(
            xf[D * q : D * q + D, :].rearrange("d (p c) -> d p c", p=2)[
                :, :, a0:a1
            ],
            xv[q : q + 1, :, :, a0:a1],
        )

    CH = [(0, 514), (514, 1538), (1538, 2562)]
    for a0, a1 in CH:
        load_chunk_q(nc.sync, 0, a0, a1)
    for a0, a1 in CH:
        load_chunk_q(nc.gpsimd, 1, a0, a1)

    # ---- ACT table warmup (exp/tanh share one table set; load it early)
    warm = sm.tile([1, 8], f32)
    nc.vector.memset(warm[:], 0.0)
    warm2 = sm.tile([1, 8], f32)
    nc.scalar.activation(warm2[:], warm[:], AF.Exp)

    # ---- PE warmup: dummy matmuls on a memset tile (no data deps) to
    # start the PE p-state ramp during the const load window.
    wsrc = sm.tile([128, 512], BF16)
    nc.vector.memset(wsrc[:], 0.0)
    wup = psum_s.tile([64, 512], f32, tag="s")
    for _ in range(3):
        nc.tensor.matmul(wup[:], wsrc[:, 0:64], wsrc[:], start=True, stop=True)

    # ---- gating from the fp32 window in cf: per q-half (batches {2q,2q+1})
    # logits[b, e] = sum_{d,t} gwin[64q+d, 5p+t] * w_gate[d*5+t, e]
    gwin = cf[:, C_GW : C_GW + 10].rearrange("d (p t) -> d t p", p=2)
    gtrs = []
    for q in range(2):
        lg = psum_s.tile([2, E], f32, tag="s", name=f"lg{q}")
        for t in range(5):
            nc.tensor.matmul(
                lg[:],
                gwin[D * q : D * q + D, t : t + 1, :],
                cf[D * q : D * q + D, C_WG + E * t : C_WG + E * t + E],
                start=(t == 0),
                stop=(t == 4),
            )
        gtrs.append(_softmax_top2(nc, sm, lg, f32, AX, OP, AF, q))
    # gT columns in PARTITION-BLOCK order j = 2p+q (batch b = 2q+p), so
    # downstream W_eff rows come out pre-permuted for the one-shot gather.
    gT = sm.tile([E, NB], FAST_DT)
    for q in range(2):
        # gtr col p -> gT col j = 2p+q (strided dest)
        nc.vector.tensor_copy(
            gT[:, q : q + 3 : 2], gtrs[q][0:E, 0:2]
        )

    # ---- W_eff / b_eff, packed for ONE DRAM-bounce gather:
    #   weff2[j, r*33 + s] (bf16): s<32 -> W_eff[b(j)][s, r]; s=32 -> b_eff
    wp3 = psum_s.tile([NB, OC], f32, tag="s")
    nc.tensor.matmul(wp3[:], gT[:], c2b[:], start=True, stop=True)
    weff2 = sm.tile([NB, OC * 33], BF16)
    wp1 = psum_s.tile([NB, 512], f32, tag="s")
    nc.tensor.matmul(wp1[:], gT[:], c2w[:, 0:512], start=True, stop=True)
    nc.vector.tensor_copy(
        weff2[:, 0 : 16 * 33].rearrange("b (r s) -> b r s", s=33)[:, :, 0:32],
        wp1[:].rearrange("b (r s) -> b r s", s=32),
    )
    wp2 = psum_s.tile([NB, 512], f32, tag="s")
    nc.tensor.matmul(wp2[:], gT[:], c2w[:, 512:1024], start=True, stop=True)
    nc.vector.tensor_copy(
        weff2[:, 16 * 33 :].rearrange("b (r s) -> b r s", s=33)[:, :, 0:32],
        wp2[:].rearrange("b (r s) -> b r s", s=32),
    )
    nc.vector.tensor_copy(
        weff2[:].rearrange("b (r s) -> b r s", s=33)[:, :, 32:33],
        wp3[:].rearrange("b (r s) -> b r s", s=1),
    )

    # DRAM bounce (SBUF APs must not cross partitions mid-dim; DRAM-side
    # reshape is free): ONE out-DMA + ONE gather into [128, 33].
    wscr = dram.tile([NB, OC * 33], BF16)
    nc.scalar.dma_start(wscr[:], weff2[:], max_dma_last_dim=OC * 33)
    wpk = const.tile([128, 33], BF16)
    nc.scalar.dma_start(
        wpk[:], wscr[:, :].rearrange("b (r s) -> (b r) s", s=33)
    )
    # block-diag combine weights + bias, assembled with same-partition ops
    weT = const.tile([128, 128], BF16)
    nc.vector.memset(weT[:], 0.0)
    for j in range(4):
        nc.vector.tensor_copy(
            weT[32 * j : 32 * j + 32, 32 * j : 32 * j + 32],
            wpk[32 * j : 32 * j + 32, 0:32],
        )
    beff = sm.tile([128, 1], f32)
    nc.vector.tensor_copy(beff[:], wpk[:, 32:33])

    # ---- last x chunk on the (now free) scalar queue
    load_chunk_q(nc.scalar, 0, 2562, 4096)
    load_chunk_q(nc.scalar, 1, 2562, 4096)

    # persistent output image: yb[64p + 32q + oc, c] = y[2q+p, oc, c], bf16
    yb = const.tile([128, LP], BF16)

    # ---- main loop over position tiles
    for t in range(NT):
        c0 = t * TS
        n = min(TS, LP - c0)
        hp = psum_h.tile([128, TS], f32, tag="hp")
        for k in range(3):
            for p in range(2):
                nc.tensor.matmul(
                    hp[64 * p : 64 * p + 64, 0:n],
                    w1t[:, 64 * k : 64 * k + 64],
                    xf[:, L * p + c0 + k : L * p + c0 + k + n],
                    start=(k == 0),
                    stop=(k == 2),
                )
        hs = hsb.tile([128, TS], BF16, tag="hs")
        nc.scalar.activation(hs[:, 0:n], hp[:, 0:n], AF.Tanh, bias=b1p, scale=1.0)
        op_ = psum_o.tile([128, TS], f32, tag="op")
        nc.tensor.matmul(op_[:, 0:n], weT[:], hs[:, 0:n], start=True, stop=True)
        # PSUM drain + b_eff add + bf16 cast, all on DVE (ACT is loaded
        # with tanhs + the const/bounce DMA issue cost)
        nc.vector.tensor_scalar(
            yb[:, c0 : c0 + n], op_[:, 0:n], beff[:], None, op0=OP.add
        )
        if t == NT // 2 - 1:
            _store_half(nc, [nc.gpsimd] * 4, y_d, yb, 0, LP // 2)
    _store_half(nc, [nc.sync, nc.sync, nc.gpsimd, nc.gpsimd], y_d, yb, LP // 2, LP)


def _store_half(nc, engs, y_d, yb, a0, a1):
    # batch b = 2q+p lives at partition block j = 2p+q
    for b in range(NB):
        p, q = b % 2, b // 2
        j = 2 * p + q
        engs[b].dma_start(
            y_d.ap()[b, :, a0:a1],
            yb[32 * j : 32 * j + 32, a0:a1],
            max_dma_last_dim=a1 - a0,
        )


def _build():
    if "nc" in _CACHE:
        return _CACHE["nc"]
    nc = bacc.Bacc(
        "TRN2",
        target_bir_lowering=False,
        debug=False,
        num_devices=NCORES,
        detect_race_conditions=False,
    )
    f32 = mybir.dt.float32
    x_d = nc.dram_tensor("x", [NB, D, L], BF16, kind="ExternalInput")
    cf_d = nc.dram_tensor("cf", [128, NCF], f32, kind="ExternalInput")
    w1_d = nc.dram_tensor("w1", [128, NW1T], BF16, kind="ExternalInput")
    c2r_d = nc.dram_tensor("c2r", [E, NC2], f32, kind="ExternalInput")
    y_d = nc.dram_tensor("y", [NB, OC, LP], BF16, kind="ExternalOutput")

    with tile.TileContext(nc) as tc:
        with ExitStack() as ctx:
            _emit(ctx, tc, nc, x_d, cf_d, w1_d, c2r_d, y_d)
    nc.compile()
    _CACHE["nc"] = nc
    return nc


def _prep_weights(w_gate, conv1_w, conv1_b, conv2_w, conv2_b):
    import ml_dtypes

    bf16 = ml_dtypes.bfloat16
    w_gate = np.asarray(w_gate, np.float32)
    conv1_w = np.asarray(conv1_w, np.float32)
    conv1_b = np.asarray(conv1_b, np.float32)
    conv2_w = np.asarray(conv2_w, np.float32)
    conv2_b = np.asarray(conv2_b, np.float32)
    # bf16 image: block-diagonal conv1 weights for 2-batch-stacked matmuls
    w1 = np.zeros((128, NW1T), bf16)
    wkt = conv1_w.transpose(1, 2, 0).astype(bf16)  # [d, k, oc]
    for k in range(3):
        w1[0:D, 64 * k : 64 * k + OC] = wkt[:, k, :]
        w1[D : 2 * D, 64 * k + OC : 64 * k + 2 * OC] = wkt[:, k, :]
    # f32 image (gating window filled per core in _run)
    cf = np.zeros((128, NCF), np.float32)
    wgr = w_gate.reshape(D, 5 * E)
    cf[0:D, C_WG : C_WG + 5 * E] = wgr
    cf[D : 2 * D, C_WG : C_WG + 5 * E] = wgr
    cf[:, C_B1P] = np.tile(conv1_b, 4)
    # fp32r conv2 image: c2w[e, ic*32+oc] = conv2_w[oc*8+e, ic, 0]
    c2 = np.zeros((E, NC2), np.float32)
    c2[:, C2_W : C2_W + OC * OC] = (
        conv2_w[:, :, 0].reshape(OC, E, OC).transpose(1, 2, 0).reshape(E, OC * OC)
    )
    c2[:, C2_B : C2_B + OC] = conv2_b.reshape(OC, E).T
    return np.ascontiguousarray(w1), cf, np.ascontiguousarray(c2)


def _run(x, w_gate, conv1_w, conv1_b, conv2_w, conv2_b, **spmd_kwargs):
    import ml_dtypes

    bf16 = ml_dtypes.bfloat16
    x = np.asarray(x, np.float32)
    assert x.shape == (B, D, L), x.shape
    w1, cf, c2 = _prep_weights(w_gate, conv1_w, conv1_b, conv2_w, conv2_b)
    xb = np.ascontiguousarray(x.astype(bf16))
    nc = _build()
    in_maps = []
    for i in range(NCORES):
        xs = x[NB * i : NB * (i + 1)]  # fp32 shard for the gating window
        cfi = cf.copy()
        # gwin[64q+d, 5p+t] = x[2q+p, d, L-6+t]
        gw = xs[:, :, L - 6 : L - 1].reshape(2, 2, D, 5)  # [q, p, d, t]
        cfi[:, C_GW : C_GW + 10] = (
            gw.transpose(0, 2, 1, 3).reshape(2 * D, 10)
        )
        in_maps.append(
            {
                "x": np.ascontiguousarray(xb[NB * i : NB * (i + 1)]),
                "cf": cfi,
                "w1": w1,
                "c2r": c2,
            }
        )
    res = bass_utils.run_bass_kernel_spmd(
        nc, in_maps, core_ids=list(range(NCORES)), **spmd_kwargs
    )
    # y[64p + 32q + oc] partition layout maps back to batch b = 2q + p
    y = np.concatenate([r["y"] for r in res.results], axis=0)
    return np.ascontiguousarray(y.astype(np.float32)), res


def kernel(x, w_gate, conv1_w, conv1_b, conv2_w, conv2_b):
    y, _ = _run(x, w_gate, conv1_w, conv1_b, conv2_w, conv2_b)
    return y
